# revision 3
# baseline (speedup 1.0000x reference)
"""Trainium2 Bass kernel for nn_Conv2DLayer_29308856828627.

Reference op: upfirdn2d 2x FIR upsample (filter outer([1,3,3,1])/8 * 4) followed
by a 3x3 "true conv" (flipped kernel), bias + lrelu(0.2) * sqrt(2), clamp +-256.

Math used here: composing the zero-insertion upsample FIR (4x4) with the 3x3
conv gives a 6x6 kernel on the upsampled grid; polyphase-decomposing it over the
4 output pixel phases (a, b) in {0,1}^2 yields 4 independent 3x3 SAME convs on
the ORIGINAL 128x128 input:

    y[n, o, 2i+a, 2j+b] = sum_{c,k,l} Wc[a,b,o,c,k,l] * x[n, c, i+k-1, j+l-1]

with Wc[a,b][k,l] = g[2k+1-a, 2l+1-b], g = conv2_full(flip(weight)*gain, 4*f).
This is exact (verified to fp32 roundoff against the jax reference).

Device layout per core (data-parallel over batch, 2 images/core on 8 cores):
  - x is host-padded to 130x130 (SAME border zeros) and cast to bf16; it is
    DMA'd twice into SBUF partitions 0-63 and 64-127 so the two PE row-groups
    can run concurrent K=64 matmuls (tile_position (0,0) and (64,0)).
  - Output channels of a matmul are m = a*64 + o (both row-phases at once,
    M=128); the two column-phases b accumulate in two separate PSUM banks.
  - Per 4-conv-row tile: 9 taps x 2 phases = 18 matmuls, N=512 each.
  - Epilogue: ACT Prelu(alpha=0.2) with folded sqrt(2) bias/scale, then DVE
    clamp(+-256) writing the b-phase column interleave; one DMA per row-phase
    writes contiguous 1KB rows back in NCHW.
"""

import math
import os
import sys

import numpy as np
import ml_dtypes

for _p in ("/opt/trn_rl_repo", "/root/.axon_site/_ro/trn_rl_repo"):
    if os.path.isdir(_p) and _p not in sys.path:
        sys.path.append(_p)

import json

import concourse.bass as bass
import concourse.bass2jax as bass2jax
import concourse.bass_utils as bass_utils
import concourse.mybir as mybir
import concourse.tile as tile
from concourse.bass_utils import run_bass_kernel_spmd
from concourse.vector_clock import ScopedClock, VectorClock
from contextlib import ExitStack

BF16 = mybir.dt.bfloat16
F32 = mybir.dt.float32

N_CORES = 8
N_IMGS = 16
IMGS_PER_CORE = N_IMGS // N_CORES
H = W = 128
C = 64
O = 64
PR = H + 2
PC = W + 2
FREE = PR * PC
NTAPS = 9
ROWS_PER_TILE = 4
N_ROW_TILES = H // ROWS_PER_TILE  # 32
SQ2 = math.sqrt(2.0)


def _split_drain_and_barrier(self, tick_clock, wait_clock):
    """Kernel-tail drain with one sync-wait per drain instruction (the TPB
    instruction format has a single wait slot; a kernel touching all 8 HWDGE
    lanes + 3 engines needs 11 waits)."""
    nc = self.nc
    gc = tick_clock.global_clock
    full = list(gc)
    nonzero = [i for i, v in enumerate(full) if v > 0]
    for i in nonzero:
        vec = [0] * len(full)
        vec[i] = full[i]
        d = nc.sync.drain()
        wait_clock.add_sem_waits(d.ins, ScopedClock({None: VectorClock(vec)}))
    nc.all_engine_barrier()
    assert self.sems is not None
    popped = nc._tile_sem_poison_stack.pop()
    assert popped is self._sem_poison
    nc.clear_and_free_semaphores(list(self.sems.allocated().values()))
    nc.all_engine_barrier()


if getattr(tile.TileContext, "_drain_patch", None) is not _split_drain_and_barrier:
    tile.TileContext._drain_and_barrier = _split_drain_and_barrier
    tile.TileContext._drain_patch = _split_drain_and_barrier


_orig_compile_bir_kernel = bass_utils.compile_bir_kernel


def _compile_with_wait_split(bir_json, tmpdir, neff_name="file.neff"):
    """This walrus build accepts a single sync-wait per instruction; Tile can
    attach several (e.g. a RAW psum wait + a WAR buffer-reuse wait). Split the
    extras onto standalone Drain instructions inserted just before the owner —
    sequencer semantics are identical."""
    j = json.loads(bir_json)
    n_new = 0
    for fn in j.get("functions", []):
        for blk in fn.get("blocks", []):
            insts = blk.get("instructions", [])
            out = []
            for inst in insts:
                si = inst.get("sync_info")
                waits = (si or {}).get("on_wait") or []
                if len(waits) > 1:
                    for w in waits[:-1]:
                        n_new += 1
                        out.append(
                            {
                                "debug": inst.get("debug", 0),
                                "engine": inst["engine"],
                                "ins": [],
                                "outs": [],
                                "is_reset_sema": False,
                                "name": f"{inst['name']}-wsplit{n_new}",
                                "opcode": "Drain",
                                "sync_info": {"on_update": [], "on_wait": [w]},
                            }
                        )
                    si["on_wait"] = [waits[-1]]
                out.append(inst)
            blk["instructions"] = out
    data = json.dumps(j).encode() if n_new else bir_json
    return _orig_compile_bir_kernel(data, tmpdir, neff_name)


if bass_utils.compile_bir_kernel is not _compile_with_wait_split:
    bass_utils.compile_bir_kernel = _compile_with_wait_split
    bass2jax.compile_bir_kernel = _compile_with_wait_split


def compose_weights(weight: np.ndarray, f: np.ndarray) -> np.ndarray:
    """Host-side exact composition of FIR-upsample + conv into per-phase 3x3
    kernels, laid out for the device matmuls: [128, 9*128] with
    W_sb[b*64+c, t*128 + a*64 + o] = sqrt(2) * Wc[a,b,o,c,t//3,t%3]."""
    WEIGHT_GAIN = 1.0 / np.sqrt(C * 9)
    wf = (weight.astype(np.float64) * WEIGHT_GAIN)[:, :, ::-1, ::-1]
    fk = f.astype(np.float64) * 4.0
    g = np.zeros((O, C, 6, 6))
    for sk in range(3):
        for sl in range(3):
            g[:, :, sk : sk + 4, sl : sl + 4] += (
                wf[:, :, sk, sl, None, None] * fk[None, None]
            )
    W_sb = np.zeros((128, NTAPS * 128))
    for b in range(2):
        for t in range(NTAPS):
            k, l = divmod(t, 3)
            for a in range(2):
                W_sb[b * 64 : b * 64 + 64, t * 128 + a * 64 : t * 128 + a * 64 + 64] = (
                    SQ2 * g[:, :, 2 * k + 1 - a, 2 * l + 1 - b].T
                )
    return W_sb


def build_nc(n_imgs: int = IMGS_PER_CORE) -> bass.Bass:
    nc = bass.Bass()
    x_in = nc.declare_dram_parameter("xpad", [n_imgs, C, FREE], BF16, isOutput=False)
    w_in = nc.declare_dram_parameter("wt", [128, NTAPS * 128], BF16, isOutput=False)
    b_in = nc.declare_dram_parameter("bias2", [128, 1], F32, isOutput=False)
    y_out = nc.declare_dram_parameter(
        "y", [n_imgs, O, 2 * H, 2 * W], F32, isOutput=True
    )

    with tile.TileContext(nc) as tc, ExitStack() as ctx:
        const_pool = ctx.enter_context(tc.tile_pool(name="const", bufs=1))
        x_pool = ctx.enter_context(tc.tile_pool(name="xp", bufs=2))
        psum_pool = ctx.enter_context(tc.tile_pool(name="ps", bufs=4, space="PSUM"))
        t_pool = ctx.enter_context(tc.tile_pool(name="tp", bufs=4))
        y_pool = ctx.enter_context(tc.tile_pool(name="yp", bufs=4))

        wt = const_pool.tile([128, NTAPS * 128], BF16)
        nc.sync.dma_start(wt[:], w_in[:])
        bias_sb = const_pool.tile([128, 1], F32)
        nc.sync.dma_start(bias_sb[:], b_in[:])
        # Pre-touch bias on the ACT engine so later activations carry only
        # their PSUM wait (single HW wait slot per instruction).
        bias_scratch = const_pool.tile([128, 1], F32)
        nc.scalar.copy(bias_scratch[:], bias_sb[:])

        for n in range(n_imgs):
            X2 = x_pool.tile([128, FREE], BF16)
            nc.sync.dma_start(X2[0:64, :], x_in[n])
            nc.sync.dma_start(X2[64:128, :], x_in[n])
            X2v = X2.rearrange("p (r c) -> p r c", r=PR)
            yv = y_out[n].rearrange("o (i a) c -> a o i c", a=2)
            for i0 in range(N_ROW_TILES):
                ps0 = psum_pool.tile([128, ROWS_PER_TILE, 128], F32, tag="ps")
                ps1 = psum_pool.tile([128, ROWS_PER_TILE, 128], F32, tag="ps")
                for t in range(NTAPS):
                    k, l = divmod(t, 3)
                    r0 = ROWS_PER_TILE * i0 + k
                    first, last = (t == 0), (t == NTAPS - 1)
                    nc.tensor.matmul(
                        ps0[:],
                        wt[0:64, t * 128 : (t + 1) * 128],
                        X2v[0:64, r0 : r0 + ROWS_PER_TILE, l : l + 128],
                        start=first,
                        stop=last,
                    )
                    nc.tensor.matmul(
                        ps1[:],
                        wt[64:128, t * 128 : (t + 1) * 128],
                        X2v[64:128, r0 : r0 + ROWS_PER_TILE, l : l + 128],
                        start=first,
                        stop=last,
                    )
                yt = y_pool.tile([128, ROWS_PER_TILE, 2 * W], F32)
                for b, ps in enumerate((ps0, ps1)):
                    tb = t_pool.tile([128, ROWS_PER_TILE, 128], F32, tag="tb")
                    nc.scalar.activation(
                        tb[:],
                        ps[:],
                        mybir.ActivationFunctionType.Prelu,
                        bias=bias_sb[:],
                        scale=1.0,
                        alpha=0.2,
                    )
                    nc.vector.tensor_scalar(
                        yt[:, :, b::2],
                        tb[:],
                        256.0,
                        -256.0,
                        op0=mybir.AluOpType.min,
                        op1=mybir.AluOpType.max,
                    )
                r = ROWS_PER_TILE * i0
                nc.sync.dma_start(yv[0, :, r : r + ROWS_PER_TILE, :], yt[0:64])
                nc.sync.dma_start(yv[1, :, r : r + ROWS_PER_TILE, :], yt[64:128])
    return nc


def prepare_inputs(x, weight, bias, f):
    W_sb = compose_weights(np.asarray(weight), np.asarray(f)).astype(
        ml_dtypes.bfloat16
    )
    bias2 = (SQ2 * np.tile(np.asarray(bias, dtype=np.float64), 2)).reshape(
        128, 1
    ).astype(np.float32)
    x = np.asarray(x, dtype=np.float32)
    n = x.shape[0]
    xpad = np.zeros((n, C, PR, PC), dtype=ml_dtypes.bfloat16)
    xpad[:, :, 1 : 1 + H, 1 : 1 + W] = x.astype(ml_dtypes.bfloat16)
    xpad = xpad.reshape(n, C, FREE)
    return xpad, W_sb, bias2


def run(x, weight, bias, f, trace=False, **spmd_kwargs):
    xpad, W_sb, bias2 = prepare_inputs(x, weight, bias, f)
    n = xpad.shape[0]
    assert n == N_IMGS, f"expected batch {N_IMGS}, got {n}"
    nc = build_nc(IMGS_PER_CORE)
    in_maps = [
        {
            "xpad": xpad[i * IMGS_PER_CORE : (i + 1) * IMGS_PER_CORE],
            "wt": W_sb,
            "bias2": bias2,
        }
        for i in range(N_CORES)
    ]
    res = run_bass_kernel_spmd(
        nc, in_maps, list(range(N_CORES)), trace=trace, **spmd_kwargs
    )
    y = np.concatenate([res.results[i]["y"] for i in range(N_CORES)], axis=0)
    return y, res


def kernel(x, weight, bias, f):
    y, _ = run(x, weight, bias, f, trace=False)
    return y


if __name__ == "__main__":
    rng = np.random.default_rng(0)
    x = rng.standard_normal((N_IMGS, C, H, W)).astype(np.float32)
    weight = rng.standard_normal((O, C, 3, 3)).astype(np.float32)
    bias = np.zeros((O,), dtype=np.float32)
    a = np.array([1.0, 3.0, 3.0, 1.0], dtype=np.float32)
    f = np.outer(a, a)
    f = f / f.sum()
    y = kernel(x, weight, bias, f)
    print("kernel output", y.shape, y.dtype)
